# revision 8
# baseline (speedup 1.0000x reference)
"""HGNN layer (hypergraph message passing) Trainium2 kernel, 8 NeuronCores.

Sharding: one graph per PAIR of cores (4 pairs run concurrently). Within
a pair, core r owns hyperedge slice S_r (contiguous 2048) and node set
R_r (two interleaved 1024-blocks: rows r*1024..(r+1)*1024 and
2048+r*1024..2048+(r+1)*1024 -- chosen so a ReduceScatter over a
contiguous half of the node axis lands each core's own rows).

Only 3 logical collectives per graph -- AllReduce(h1b+z, bf16),
AllReduce(h1d, bf16), ReduceScatter(out, fp32) -- each SPLIT INTO TWO
HALVES (6 cc ops total), and every cc op carries all 4 pairs as
concurrent replica groups. Each half-collective is triggered as soon as
its half of the producing stage finishes and its latency hides behind
the other half's compute, so the tensor engine never starves.

H ships in fp8e4 (exactly 0/1 -> lossless) in three stream layouts;
Dv/De in bf16. All big operands stream through SBUF in 1-2MB contiguous
HWDGE chunks ordered exactly as consumed; nothing big is resident. All
heavy matmuls use N=512 moving operands (PE-transposes fix orientation
where the collectives need row-major).
"""

import numpy as np
import ml_dtypes

B, N, E, D = 4, 4096, 4096, 128
NCORES = 8
HS = 2048                # per-core slice of E; also total owned nodes
LH = HS // 128           # 16 local tiles
NT = N // 128            # 32 tiles over a full 4096 dim
GROUPS = [[0, 1], [2, 3], [4, 5], [6, 7]]

_CACHE = {}


def _build():
    import concourse.bacc as bacc
    import concourse.mybir as mybir
    import concourse.tile as tile
    from concourse.masks import make_identity
    from contextlib import ExitStack

    fp32 = mybir.dt.float32
    bf16 = mybir.dt.bfloat16
    f8 = mybir.dt.float8e4
    Act = mybir.ActivationFunctionType
    Alu = mybir.AluOpType

    nc = bacc.Bacc("TRN2", target_bir_lowering=False, debug=False,
                   num_devices=NCORES)

    eps_d = nc.dram_tensor("eps", [D, 1], fp32, kind="ExternalInput")
    sbn_d = nc.dram_tensor("sbn", [D, 1], fp32, kind="ExternalInput")
    tbn_d = nc.dram_tensor("tbn", [D, 1], fp32, kind="ExternalInput")
    xwv_d = nc.dram_tensor("xwv", [D, N], bf16, kind="ExternalInput")
    attn_d = nc.dram_tensor("attn", [1, HS], fp32, kind="ExternalInput")
    ha_d = nc.dram_tensor("ha", [D, 4 * NT * 512], f8, kind="ExternalInput")
    hbm_d = nc.dram_tensor("hbm", [D, 8 * LH * 512], f8, kind="ExternalInput")
    hcm_d = nc.dram_tensor("hcm", [D, 8 * LH * 512], f8, kind="ExternalInput")
    dvt_d = nc.dram_tensor("dvt", [D, NT * HS], bf16, kind="ExternalInput")
    det_d = nc.dram_tensor("det", [D, NT * HS], bf16, kind="ExternalInput")
    y_d = nc.dram_tensor("y", [D, HS], fp32, kind="ExternalOutput")

    with tile.TileContext(nc) as tc, ExitStack() as ctx:
        const = ctx.enter_context(tc.tile_pool(name="const", bufs=1))
        mv = ctx.enter_context(tc.tile_pool(name="mv", bufs=4))
        one = ctx.enter_context(tc.tile_pool(name="one", bufs=1))
        ev = ctx.enter_context(tc.tile_pool(name="ev", bufs=4))
        ps_acc = ctx.enter_context(tc.tile_pool(name="ps_acc", bufs=4, space="PSUM"))
        ps_st = ctx.enter_context(tc.tile_pool(name="ps_st", bufs=2, space="PSUM"))
        ps_t = ctx.enter_context(tc.tile_pool(name="ps_t", bufs=2, space="PSUM"))
        dram = ctx.enter_context(tc.tile_pool(name="dram", bufs=1, space="DRAM"))

        ident = const.tile([128, 128], fp32)
        make_identity(nc, ident)
        one11 = const.tile([1, 1], fp32)
        nc.vector.memset(one11[:], 1.0)
        eps_t = const.tile([D, 1], fp32)
        nc.sync.dma_start(out=eps_t[:], in_=eps_d.ap())
        sbn_t = const.tile([D, 1], fp32)
        nc.sync.dma_start(out=sbn_t[:], in_=sbn_d.ap())
        tbn_t = const.tile([D, 1], fp32)
        nc.sync.dma_start(out=tbn_t[:], in_=tbn_d.ap())

        x_wv = one.tile([D, N], bf16, tag="x_wv")
        nc.sync.dma_start(out=x_wv[:], in_=xwv_d.ap())
        attn_t = one.tile([1, HS], fp32, tag="attn_t")
        nc.sync.dma_start(out=attn_t[:], in_=attn_d.ap())

        # ---- S2: hxT [128d, 2048e], e-chunk outer --------------------
        hxT = one.tile([D, HS], fp32, tag="hxT")
        for ec in range(4):
            ch = mv.tile([D, NT * 512], f8, tag="mv", name=f"ha{ec}")
            nc.sync.dma_start(
                out=ch[:], in_=ha_d.ap()[:, ec * NT * 512:(ec + 1) * NT * 512])
            hx = ps_acc.tile([128, 512], fp32, tag="acc", name=f"hx{ec}")
            for k in range(NT):
                rhs = ch[:, k * 512:(k + 1) * 512]
                nc.tensor.matmul(hx[:], x_wv[:, k * 128:(k + 1) * 128], rhs,
                                 start=(k == 0), stop=(k == NT - 1))
            nc.vector.tensor_copy(hxT[:, ec * 512:(ec + 1) * 512], hx[:])

        attnv = one.tile([128, LH], fp32, tag="attnv")
        for t in range(LH):
            p = ps_t.tile([128, 1], fp32, tag="pst", name=f"at{t}")
            nc.tensor.matmul(p[:], attn_t[:, t * 128:(t + 1) * 128], one11[:],
                             start=True, stop=True)
            nc.vector.tensor_copy(attnv[:, t:t + 1], p[:])

        # h1a = attn * hx (e-tiles); ehx = eps * hx (e-tiles)
        h1a = one.tile([128, HS], bf16, tag="h1a")
        ehx = one.tile([128, HS], bf16, tag="ehx")
        for t in range(LH):
            p = ps_t.tile([128, 128], fp32, tag="pst", name=f"ta{t}")
            nc.tensor.transpose(p[:], hxT[:, t * 128:(t + 1) * 128], ident[:])
            nc.vector.tensor_scalar_mul(h1a[:, t * 128:(t + 1) * 128], p[:],
                                        attnv[:, t:t + 1])
            nc.vector.tensor_scalar_mul(ehx[:, t * 128:(t + 1) * 128], p[:],
                                        eps_t[:])

        # ---- S6: h1b partial; halves -> AR1a/AR1b --------------------
        # out for n-chunk nc_ = h1a.T @ Hbm  -> [128d, 512n]; transpose
        # per 128-block into bf16 [n_l, d] image for the AllReduce.
        out2 = one.tile([D, N], bf16, tag="out2")

        def s6_half(h, cc_sb):
            for c in range(4):
                nc_i = h * 4 + c
                ch = mv.tile([D, LH * 512], f8, tag="mv", name=f"hbm{nc_i}")
                nc.sync.dma_start(
                    out=ch[:],
                    in_=hbm_d.ap()[:, nc_i * LH * 512:(nc_i + 1) * LH * 512])
                po = ps_acc.tile([128, 512], fp32, tag="acc", name=f"p6_{nc_i}")
                for j in range(LH):
                    nc.tensor.matmul(po[:], h1a[:, j * 128:(j + 1) * 128],
                                     ch[:, j * 512:(j + 1) * 512],
                                     start=(j == 0), stop=(j == LH - 1))
                stg = ev.tile([128, 512], fp32, tag="ev", name=f"e6_{nc_i}")
                nc.vector.tensor_copy(stg[:], po[:])
                for q in range(4):
                    pt = ps_t.tile([128, 128], fp32, tag="pst",
                                   name=f"t6_{nc_i}_{q}")
                    nc.tensor.transpose(pt[:], stg[:, q * 128:(q + 1) * 128],
                                        ident[:])
                    col = (c * 4 + q) * 128
                    nc.vector.tensor_copy(cc_sb[:, col:col + 128], pt[:])

        cc1a_sb = one.tile([D, HS], bf16, tag="cc1a")
        s6_half(0, cc1a_sb)
        cc1a_in = dram.tile([D, HS], bf16, tag="cc1ai")
        cc1a_out = dram.tile([D, HS], bf16, tag="cc1ao")
        nc.sync.dma_start(out=cc1a_in[:], in_=cc1a_sb[:])
        nc.gpsimd.collective_compute(
            "AllReduce", Alu.add, replica_groups=GROUPS,
            ins=[cc1a_in.opt()], outs=[cc1a_out.opt()])

        cc1b_sb = one.tile([D, HS], bf16, tag="cc1b")
        s6_half(1, cc1b_sb)
        cc1b_in = dram.tile([D, HS], bf16, tag="cc1bi")
        cc1b_out = dram.tile([D, HS], bf16, tag="cc1bo")
        nc.sync.dma_start(out=cc1b_in[:], in_=cc1b_sb[:])
        nc.gpsimd.collective_compute(
            "AllReduce", Alu.add, replica_groups=GROUPS,
            ins=[cc1b_in.opt()], outs=[cc1b_out.opt()])

        # out2 = eps * (H @ hx) partial -- independent of the AllReduces,
        # so this pass executes while AR1a/AR1b are in flight.
        for c2 in range(8):
            ch2 = mv.tile([D, LH * 512], f8, tag="mv", name=f"hbo{c2}")
            nc.sync.dma_start(
                out=ch2[:],
                in_=hbm_d.ap()[:, c2 * LH * 512:(c2 + 1) * LH * 512])
            po2 = ps_acc.tile([128, 512], fp32, tag="acc", name=f"q6_{c2}")
            for j in range(LH):
                nc.tensor.matmul(po2[:], ehx[:, j * 128:(j + 1) * 128],
                                 ch2[:, j * 512:(j + 1) * 512],
                                 start=(j == 0), stop=(j == LH - 1))
            nc.vector.tensor_copy(out2[:, c2 * 512:(c2 + 1) * 512], po2[:])

        # ---- S7: h1cT [128d, 2048r]; consume AR1 halves --------------
        h1bv = one.tile([D, N], bf16, tag="h1bv")
        nc.sync.dma_start(out=h1bv[:, 0:HS], in_=cc1a_out[:])

        hc_ps = [ps_acc.tile([128, 512], fp32, tag="acc", name=f"hc{rc}")
                 for rc in range(4)]
        for kh in range(2):
            if kh == 1:
                nc.sync.dma_start(out=h1bv[:, HS:N], in_=cc1b_out[:])
            for rc in range(4):
                ch = mv.tile([D, LH * 512], bf16, tag="mv",
                             name=f"dv{kh}_{rc}")
                off = (kh * 4 + rc) * LH * 512
                nc.sync.dma_start(out=ch[:],
                                  in_=dvt_d.ap()[:, off:off + LH * 512])
                for k in range(LH):
                    kg = kh * LH + k
                    nc.tensor.matmul(hc_ps[rc][:],
                                     h1bv[:, kg * 128:(kg + 1) * 128],
                                     ch[:, k * 512:(k + 1) * 512],
                                     start=(kg == 0), stop=(kg == NT - 1))
        h1cT = one.tile([D, HS], fp32, tag="hxT", name="h1cT")
        for rc in range(4):
            nc.vector.tensor_copy(h1cT[:, rc * 512:(rc + 1) * 512],
                                  hc_ps[rc][:])
        h1cv = one.tile([128, HS], bf16, tag="h1a", name="h1cv")
        for t in range(LH):
            p = ps_t.tile([128, 128], fp32, tag="pst", name=f"t7{t}")
            nc.tensor.transpose(p[:], h1cT[:, t * 128:(t + 1) * 128],
                                ident[:])
            nc.vector.tensor_copy(h1cv[:, t * 128:(t + 1) * 128], p[:])

        # ---- S8: h1d partial over ALL e; halves -> AR2a/AR2b ---------
        def s8_half(h, cc_sb):
            for c in range(4):
                ec = h * 4 + c
                ch = mv.tile([D, LH * 512], f8, tag="mv", name=f"hcm{ec}")
                nc.sync.dma_start(
                    out=ch[:],
                    in_=hcm_d.ap()[:, ec * LH * 512:(ec + 1) * LH * 512])
                po = ps_acc.tile([128, 512], fp32, tag="acc", name=f"p8_{ec}")
                for tau in range(LH):
                    nc.tensor.matmul(po[:], h1cv[:, tau * 128:(tau + 1) * 128],
                                     ch[:, tau * 512:(tau + 1) * 512],
                                     start=(tau == 0), stop=(tau == LH - 1))
                stg = ev.tile([128, 512], fp32, tag="ev", name=f"e8_{ec}")
                nc.vector.tensor_copy(stg[:], po[:])
                for q in range(4):
                    pt = ps_t.tile([128, 128], fp32, tag="pst",
                                   name=f"t8_{ec}_{q}")
                    nc.tensor.transpose(pt[:], stg[:, q * 128:(q + 1) * 128],
                                        ident[:])
                    col = (c * 4 + q) * 128
                    nc.vector.tensor_copy(cc_sb[:, col:col + 128], pt[:])

        cc2a_sb = one.tile([D, HS], bf16, tag="cc2a")
        s8_half(0, cc2a_sb)
        cc2a_in = dram.tile([D, HS], bf16, tag="cc2ai")
        cc2a_out = dram.tile([D, HS], bf16, tag="cc2ao")
        nc.sync.dma_start(out=cc2a_in[:], in_=cc2a_sb[:])
        nc.gpsimd.collective_compute(
            "AllReduce", Alu.add, replica_groups=GROUPS,
            ins=[cc2a_in.opt()], outs=[cc2a_out.opt()])
        cc2b_sb = one.tile([D, HS], bf16, tag="cc2b")
        s8_half(1, cc2b_sb)
        cc2b_in = dram.tile([D, HS], bf16, tag="cc2bi")
        cc2b_out = dram.tile([D, HS], bf16, tag="cc2bo")
        nc.sync.dma_start(out=cc2b_in[:], in_=cc2b_sb[:])
        nc.gpsimd.collective_compute(
            "AllReduce", Alu.add, replica_groups=GROUPS,
            ins=[cc2b_in.opt()], outs=[cc2b_out.opt()])

        # ---- S9: h1eT [128d, 2048s]; consume AR2 halves; + eps*hxT ---
        he_ps = [ps_acc.tile([128, 512], fp32, tag="acc", name=f"he{sc}")
                 for sc in range(4)]
        h1d_sb = one.tile([D, N], bf16, tag="h1bv", name="h1d_sb")
        for eh in range(2):
            cco = cc2a_out if eh == 0 else cc2b_out
            nc.sync.dma_start(out=h1d_sb[:, eh * HS:(eh + 1) * HS],
                              in_=cco[:])
            for sc in range(4):
                ch = mv.tile([D, LH * 512], bf16, tag="mv",
                             name=f"de{eh}_{sc}")
                off = (eh * 4 + sc) * LH * 512
                nc.sync.dma_start(out=ch[:],
                                  in_=det_d.ap()[:, off:off + LH * 512])
                for t in range(LH):
                    tg = eh * LH + t
                    nc.tensor.matmul(he_ps[sc][:],
                                     h1d_sb[:, tg * 128:(tg + 1) * 128],
                                     ch[:, t * 512:(t + 1) * 512],
                                     start=(tg == 0), stop=(tg == NT - 1))
        hT = one.tile([D, HS], fp32, tag="hT", name="hT")
        for sc in range(4):
            nc.vector.tensor_copy(hT[:, sc * 512:(sc + 1) * 512],
                                  he_ps[sc][:])
        hv = one.tile([128, HS], bf16, tag="hv")
        for t in range(LH):
            p = ps_t.tile([128, 128], fp32, tag="pst", name=f"t10{t}")
            nc.tensor.transpose(p[:], hT[:, t * 128:(t + 1) * 128], ident[:])
            nc.vector.tensor_copy(hv[:, t * 128:(t + 1) * 128], p[:])

        # ---- S11: out partial [n, d]; halves -> RSa/RSb --------------
        def s11_half(h, cc_in):
            for c in range(4):
                nc_i = h * 4 + c
                ch = mv.tile([D, LH * 512], f8, tag="mv", name=f"hbm2_{nc_i}")
                nc.sync.dma_start(
                    out=ch[:],
                    in_=hbm_d.ap()[:, nc_i * LH * 512:(nc_i + 1) * LH * 512])
                po = ps_acc.tile([128, 512], fp32, tag="acc",
                                 name=f"p11_{nc_i}")
                for j in range(LH):
                    nc.tensor.matmul(po[:], hv[:, j * 128:(j + 1) * 128],
                                     ch[:, j * 512:(j + 1) * 512],
                                     start=(j == 0), stop=(j == LH - 1))
                stg = ev.tile([128, 512], fp32, tag="ev", name=f"e11_{nc_i}")
                nc.vector.tensor_tensor(
                    stg[:], po[:], out2[:, nc_i * 512:(nc_i + 1) * 512],
                    op=Alu.add)
                for q in range(4):
                    pt = ps_t.tile([128, 128], fp32, tag="pst",
                                   name=f"t11_{nc_i}_{q}")
                    nc.tensor.transpose(pt[:], stg[:, q * 128:(q + 1) * 128],
                                        ident[:])
                    eo = ev.tile([128, 128], fp32, tag="ev2",
                                 name=f"eo11_{nc_i}_{q}")
                    nc.vector.tensor_copy(eo[:], pt[:])
                    row = (c * 4 + q) * 128
                    nc.sync.dma_start(out=cc_in.opt()[row:row + 128, :],
                                      in_=eo[:])

        cc4a_in = dram.tile([HS, D], fp32, tag="cc4ai")
        cc4a_out = dram.tile([HS // 2, D], fp32, tag="cc4ao")
        s11_half(0, cc4a_in)
        nc.gpsimd.collective_compute(
            "ReduceScatter", Alu.add, replica_groups=GROUPS,
            ins=[cc4a_in.opt()], outs=[cc4a_out.opt()])
        cc4b_in = dram.tile([HS, D], fp32, tag="cc4bi")
        cc4b_out = dram.tile([HS // 2, D], fp32, tag="cc4bo")
        s11_half(1, cc4b_in)
        nc.gpsimd.collective_compute(
            "ReduceScatter", Alu.add, replica_groups=GROUPS,
            ins=[cc4b_in.opt()], outs=[cc4b_out.opt()])

        # ---- S12: epilogue bn(lrelu(out)) per RS half ----------------
        y_sb = one.tile([D, HS], fp32, tag="y_sb")
        for h, cco in ((0, cc4a_out), (1, cc4b_out)):
            rs_sb = one.tile([128, HS // 2], fp32, tag="rs_sb",
                             name=f"rs{h}")
            nc.sync.dma_start(
                out=rs_sb[:].rearrange("p (t d) -> p t d", t=8),
                in_=cco.opt().rearrange("(t p) d -> p t d", p=128))
            for t in range(8):
                p = ps_t.tile([128, 128], fp32, tag="pst", name=f"t12{h}_{t}")
                nc.tensor.transpose(p[:], rs_sb[:, t * 128:(t + 1) * 128],
                                    ident[:])
                col = h * (HS // 2) + t * 128
                nc.scalar.activation(y_sb[:, col:col + 128], p[:],
                                     Act.Lrelu, alpha=0.01)
        nc.vector.tensor_scalar(y_sb[:], y_sb[:], sbn_t[:], tbn_t[:],
                                op0=Alu.mult, op1=Alu.add)
        nc.sync.dma_start(out=y_d.ap(), in_=y_sb[:])

    nc.finalize()
    return nc


def _get_nc():
    if "nc" not in _CACHE:
        _CACHE["nc"] = _build()
    return _CACHE["nc"]


def _rows(r):
    """Node rows owned by pair-role r: two interleaved 1024-blocks."""
    return np.r_[r * 1024:(r + 1) * 1024, 2048 + r * 1024:2048 + (r + 1) * 1024]


def _shard(inputs):
    f8 = ml_dtypes.float8_e4m3
    bf16 = ml_dtypes.bfloat16
    H = np.asarray(inputs["incident_mat"], dtype=np.float32)
    Dvm = np.asarray(inputs["degree_v"], dtype=np.float32)
    Dem = np.asarray(inputs["degree_e"], dtype=np.float32)
    x = np.asarray(inputs["x"], dtype=np.float32)
    em = np.asarray(inputs["e_masks"])
    W = np.asarray(inputs["mlp_W"], dtype=np.float32)
    bv = np.asarray(inputs["mlp_b"], dtype=np.float32)
    th = np.asarray(inputs["theta_att"], dtype=np.float32)
    eps = float(np.asarray(inputs["eps"]).reshape(-1)[0])
    bng = np.asarray(inputs["bn_gamma"], dtype=np.float32)
    bnb = np.asarray(inputs["bn_beta"], dtype=np.float32)
    bnm = np.asarray(inputs["bn_mean"], dtype=np.float32)
    bnv = np.asarray(inputs["bn_var"], dtype=np.float32)

    s_bn = (bng / np.sqrt(bnv + 1e-5)).reshape(D, 1).astype(np.float32)
    t_bn = (bnb - bnm * s_bn[:, 0]).reshape(D, 1).astype(np.float32)

    com = {
        "eps": np.full((D, 1), eps, dtype=np.float32),
        "sbn": np.ascontiguousarray(s_bn),
        "tbn": np.ascontiguousarray(t_bn),
    }
    xwv_g, attn_g = [], []
    for g in range(B):
        xw = (x[g] @ W + bv).astype(np.float32)
        xwv_g.append(np.ascontiguousarray(
            xw.reshape(NT, 128, D).transpose(1, 0, 2)
            .reshape(128, N).astype(bf16)))
        s = H[g].T @ (x[g] @ th[:, 0])
        s = np.where(em[g] == 0, -np.inf, s)
        a = np.exp(s - s.max())
        a = (a / a.sum()).astype(np.float32)
        attn_g.append(a)

    in_maps = []
    for c in range(NCORES):
        g, r = c // 2, c % 2
        e0 = r * HS
        rows = _rows(r)
        Hg = H[g].astype(f8)
        m = dict(com)
        m["xwv"] = xwv_g[g]
        m["attn"] = np.ascontiguousarray(attn_g[g][e0:e0 + HS].reshape(1, HS))
        # ha: [p, ec*NT*512 + k*512 + e_l] = H[k*128+p, e0 + ec*512 + e_l]
        A = Hg[:, e0:e0 + HS]
        m["ha"] = np.ascontiguousarray(
            A.reshape(NT, 128, 4, 512).transpose(1, 2, 0, 3)
            .reshape(128, 4 * NT * 512))
        # hbm: [p, nc*LH*512 + j*512 + n_l] = H[nc*512+n_l, e0 + j*128 + p]
        Bm = np.ascontiguousarray(A.T)       # [2048 e, 4096 n]
        m["hbm"] = np.ascontiguousarray(
            Bm.reshape(LH, 128, 8, 512).transpose(1, 2, 0, 3)
            .reshape(128, 8 * LH * 512))
        # hcm: [p, ec*LH*512 + tau*512 + e_l] = H[rows[tau*128+p], ec*512+e_l]
        Cm = Hg[rows, :]                     # [2048 n_l, 4096 e]
        m["hcm"] = np.ascontiguousarray(
            Cm.reshape(LH, 128, 8, 512).transpose(1, 2, 0, 3)
            .reshape(128, 8 * LH * 512))
        # dvt: [p, (kh*4+rc)*LH*512 + k*512 + r_l]
        #    = Dv[rows[rc*512+r_l], (kh*LH+k)*128 + p]
        Dm = np.ascontiguousarray(Dvm[g][rows, :].T.astype(bf16))  # [4096, 2048]
        m["dvt"] = np.ascontiguousarray(
            Dm.reshape(2, LH, 128, 4, 512).transpose(2, 0, 3, 1, 4)
            .reshape(128, NT * HS))
        # det: [p, (eh*4+sc)*LH*512 + t*512 + s_l]
        #    = De[e0 + sc*512 + s_l, (eh*LH+t)*128 + p]
        Em = np.ascontiguousarray(Dem[g][e0:e0 + HS, :].T.astype(bf16))
        m["det"] = np.ascontiguousarray(
            Em.reshape(2, LH, 128, 4, 512).transpose(2, 0, 3, 1, 4)
            .reshape(128, NT * HS))
        in_maps.append(m)
    return in_maps


def kernel(**inputs):
    from concourse.bass_utils import run_bass_kernel_spmd

    nc = _get_nc()
    in_maps = _shard(inputs)
    res = run_bass_kernel_spmd(nc, in_maps, list(range(NCORES)))
    out = np.empty((B, N, D), dtype=np.float32)
    for c in range(NCORES):
        g, r = c // 2, c % 2
        yc = res.results[c]["y"]             # [128 d, 2048] image
        blk = yc.reshape(D, LH, 128).transpose(1, 2, 0).reshape(HS, D)
        out[g, _rows(r), :] = blk
    return out


# revision 9
# speedup vs baseline: 1.0314x; 1.0314x over previous
"""HGNN layer (hypergraph message passing) Trainium2 kernel, 8 NeuronCores.

Sharding: one graph per PAIR of cores (4 pairs run concurrently). Within
a pair, core r owns hyperedge slice S_r (contiguous 2048) and node set
R_r (two interleaved 1024-blocks: rows r*1024..(r+1)*1024 and
2048+r*1024..2048+(r+1)*1024 -- chosen so a ReduceScatter over a
contiguous half of the node axis lands each core's own rows).

Only 3 logical collectives per graph -- AllReduce(h1b+z, bf16),
AllReduce(h1d, bf16), ReduceScatter(out, fp32) -- each SPLIT INTO TWO
HALVES (6 cc ops total), and every cc op carries all 4 pairs as
concurrent replica groups. Each half-collective is triggered as soon as
its half of the producing stage finishes and its latency hides behind
the other half's compute, so the tensor engine never starves.

H ships in fp8e4 (exactly 0/1 -> lossless) in three stream layouts;
Dv/De in bf16. All big operands stream through SBUF in 1-2MB contiguous
HWDGE chunks ordered exactly as consumed; nothing big is resident. All
heavy matmuls use N=512 moving operands (PE-transposes fix orientation
where the collectives need row-major).
"""

import numpy as np
import ml_dtypes

B, N, E, D = 4, 4096, 4096, 128
NCORES = 8
HS = 2048                # per-core slice of E; also total owned nodes
LH = HS // 128           # 16 local tiles
NT = N // 128            # 32 tiles over a full 4096 dim
GROUPS = [[0, 1], [2, 3], [4, 5], [6, 7]]

_CACHE = {}


def _build():
    import concourse.bacc as bacc
    import concourse.mybir as mybir
    import concourse.tile as tile
    from concourse.masks import make_identity
    from contextlib import ExitStack

    fp32 = mybir.dt.float32
    bf16 = mybir.dt.bfloat16
    f8 = mybir.dt.float8e4
    Act = mybir.ActivationFunctionType
    Alu = mybir.AluOpType

    nc = bacc.Bacc("TRN2", target_bir_lowering=False, debug=False,
                   num_devices=NCORES)

    eps_d = nc.dram_tensor("eps", [D, 1], fp32, kind="ExternalInput")
    sbn_d = nc.dram_tensor("sbn", [D, 1], fp32, kind="ExternalInput")
    tbn_d = nc.dram_tensor("tbn", [D, 1], fp32, kind="ExternalInput")
    xwv_d = nc.dram_tensor("xwv", [D, N], bf16, kind="ExternalInput")
    attn_d = nc.dram_tensor("attn", [1, HS], fp32, kind="ExternalInput")
    ha_d = nc.dram_tensor("ha", [D, 4 * NT * 512], f8, kind="ExternalInput")
    hbm_d = nc.dram_tensor("hbm", [D, 8 * LH * 512], f8, kind="ExternalInput")
    hcm_d = nc.dram_tensor("hcm", [D, 8 * LH * 512], f8, kind="ExternalInput")
    dvt_d = nc.dram_tensor("dvt", [D, NT * HS], bf16, kind="ExternalInput")
    det_d = nc.dram_tensor("det", [D, NT * HS], bf16, kind="ExternalInput")
    y_d = nc.dram_tensor("y", [D, HS], fp32, kind="ExternalOutput")

    with tile.TileContext(nc) as tc, ExitStack() as ctx:
        const = ctx.enter_context(tc.tile_pool(name="const", bufs=1))
        mv = ctx.enter_context(tc.tile_pool(name="mv", bufs=4))
        one = ctx.enter_context(tc.tile_pool(name="one", bufs=1))
        ev = ctx.enter_context(tc.tile_pool(name="ev", bufs=4))
        ps_acc = ctx.enter_context(tc.tile_pool(name="ps_acc", bufs=4, space="PSUM"))
        ps_st = ctx.enter_context(tc.tile_pool(name="ps_st", bufs=2, space="PSUM"))
        ps_t = ctx.enter_context(tc.tile_pool(name="ps_t", bufs=2, space="PSUM"))
        dram = ctx.enter_context(tc.tile_pool(name="dram", bufs=1, space="DRAM"))

        ident = const.tile([128, 128], fp32)
        make_identity(nc, ident)
        one11 = const.tile([1, 1], fp32)
        nc.vector.memset(one11[:], 1.0)
        eps_t = const.tile([D, 1], fp32)
        nc.sync.dma_start(out=eps_t[:], in_=eps_d.ap())
        sbn_t = const.tile([D, 1], fp32)
        nc.sync.dma_start(out=sbn_t[:], in_=sbn_d.ap())
        tbn_t = const.tile([D, 1], fp32)
        nc.sync.dma_start(out=tbn_t[:], in_=tbn_d.ap())

        x_wv = one.tile([D, N], bf16, tag="x_wv")
        nc.sync.dma_start(out=x_wv[:], in_=xwv_d.ap())
        attn_t = one.tile([1, HS], fp32, tag="attn_t")
        nc.sync.dma_start(out=attn_t[:], in_=attn_d.ap())

        # ---- S2: hxT [128d, 2048e], e-chunk outer --------------------
        hxT = one.tile([D, HS], fp32, tag="hxT")
        for ec in range(4):
            ch = mv.tile([D, NT * 512], f8, tag="mv", name=f"ha{ec}")
            nc.sync.dma_start(
                out=ch[:], in_=ha_d.ap()[:, ec * NT * 512:(ec + 1) * NT * 512])
            hx = ps_acc.tile([128, 512], fp32, tag="acc", name=f"hx{ec}")
            for k in range(NT):
                rhs = ch[:, k * 512:(k + 1) * 512]
                nc.tensor.matmul(hx[:], x_wv[:, k * 128:(k + 1) * 128], rhs,
                                 start=(k == 0), stop=(k == NT - 1))
            nc.vector.tensor_copy(hxT[:, ec * 512:(ec + 1) * 512], hx[:])

        attnv = one.tile([128, LH], fp32, tag="attnv")
        for t in range(LH):
            p = ps_t.tile([128, 1], fp32, tag="pst", name=f"at{t}")
            nc.tensor.matmul(p[:], attn_t[:, t * 128:(t + 1) * 128], one11[:],
                             start=True, stop=True)
            nc.vector.tensor_copy(attnv[:, t:t + 1], p[:])

        # h1a = attn * hx (e-tiles); ehx = eps * hx (e-tiles)
        h1a = one.tile([128, HS], bf16, tag="h1a")
        ehx = one.tile([128, HS], bf16, tag="ehx")
        for t in range(LH):
            p = ps_t.tile([128, 128], fp32, tag="pst", name=f"ta{t}")
            nc.tensor.transpose(p[:], hxT[:, t * 128:(t + 1) * 128], ident[:])
            nc.vector.tensor_scalar_mul(h1a[:, t * 128:(t + 1) * 128], p[:],
                                        attnv[:, t:t + 1])
            nc.vector.tensor_scalar_mul(ehx[:, t * 128:(t + 1) * 128], p[:],
                                        eps_t[:])

        # ---- S6: h1b partial; halves -> AR1a/AR1b --------------------
        # out for n-chunk nc_ = h1a.T @ Hbm  -> [128d, 512n]; transpose
        # per 128-block into bf16 [n_l, d] image for the AllReduce.
        out2 = one.tile([D, N], bf16, tag="out2")

        def s6_half(h, cc_sb):
            for c in range(4):
                nc_i = h * 4 + c
                ch = mv.tile([D, LH * 512], f8, tag="mv", name=f"hbm{nc_i}")
                nc.sync.dma_start(
                    out=ch[:],
                    in_=hbm_d.ap()[:, nc_i * LH * 512:(nc_i + 1) * LH * 512])
                po = ps_acc.tile([128, 512], fp32, tag="acc", name=f"p6_{nc_i}")
                for j in range(LH):
                    nc.tensor.matmul(po[:], h1a[:, j * 128:(j + 1) * 128],
                                     ch[:, j * 512:(j + 1) * 512],
                                     start=(j == 0), stop=(j == LH - 1))
                stg = ev.tile([128, 512], fp32, tag="ev", name=f"e6_{nc_i}")
                nc.vector.tensor_copy(stg[:], po[:])
                for q in range(4):
                    pt = ps_t.tile([128, 128], fp32, tag="pst",
                                   name=f"t6_{nc_i}_{q}")
                    nc.tensor.transpose(pt[:], stg[:, q * 128:(q + 1) * 128],
                                        ident[:])
                    col = (nc_i * 4 + q) * 128
                    nc.vector.tensor_copy(cc_sb[:, col:col + 128], pt[:])

        cc1_sb = one.tile([D, N], bf16, tag="cc1a")
        s6_half(0, cc1_sb)
        s6_half(1, cc1_sb)
        cc1_in = dram.tile([D, N], bf16, tag="cc1ai")
        cc1_out = dram.tile([D, N], bf16, tag="cc1ao")
        nc.sync.dma_start(out=cc1_in[:], in_=cc1_sb[:])
        nc.gpsimd.collective_compute(
            "AllReduce", Alu.add, replica_groups=GROUPS,
            ins=[cc1_in.opt()], outs=[cc1_out.opt()])

        # out2 = eps * (H @ hx) partial -- independent of the AllReduces,
        # so this pass executes while AR1a/AR1b are in flight.
        for c2 in range(8):
            ch2 = mv.tile([D, LH * 512], f8, tag="mv", name=f"hbo{c2}")
            nc.sync.dma_start(
                out=ch2[:],
                in_=hbm_d.ap()[:, c2 * LH * 512:(c2 + 1) * LH * 512])
            po2 = ps_acc.tile([128, 512], fp32, tag="acc", name=f"q6_{c2}")
            for j in range(LH):
                nc.tensor.matmul(po2[:], ehx[:, j * 128:(j + 1) * 128],
                                 ch2[:, j * 512:(j + 1) * 512],
                                 start=(j == 0), stop=(j == LH - 1))
            nc.vector.tensor_copy(out2[:, c2 * 512:(c2 + 1) * 512], po2[:])

        # ---- S7: h1cT [128d, 2048r]; consume AR1 halves --------------
        h1bv = one.tile([D, N], bf16, tag="h1bv")
        nc.sync.dma_start(out=h1bv[:], in_=cc1_out[:])

        hc_ps = [ps_acc.tile([128, 512], fp32, tag="acc", name=f"hc{rc}")
                 for rc in range(4)]
        for kh in range(2):
            for rc in range(4):
                ch = mv.tile([D, LH * 512], bf16, tag="mv",
                             name=f"dv{kh}_{rc}")
                off = (kh * 4 + rc) * LH * 512
                nc.sync.dma_start(out=ch[:],
                                  in_=dvt_d.ap()[:, off:off + LH * 512])
                for k in range(LH):
                    kg = kh * LH + k
                    nc.tensor.matmul(hc_ps[rc][:],
                                     h1bv[:, kg * 128:(kg + 1) * 128],
                                     ch[:, k * 512:(k + 1) * 512],
                                     start=(kg == 0), stop=(kg == NT - 1))
        h1cT = one.tile([D, HS], fp32, tag="hxT", name="h1cT")
        for rc in range(4):
            nc.vector.tensor_copy(h1cT[:, rc * 512:(rc + 1) * 512],
                                  hc_ps[rc][:])
        h1cv = one.tile([128, HS], bf16, tag="h1a", name="h1cv")
        for t in range(LH):
            p = ps_t.tile([128, 128], fp32, tag="pst", name=f"t7{t}")
            nc.tensor.transpose(p[:], h1cT[:, t * 128:(t + 1) * 128],
                                ident[:])
            nc.vector.tensor_copy(h1cv[:, t * 128:(t + 1) * 128], p[:])

        # ---- S8: h1d partial over ALL e; halves -> AR2a/AR2b ---------
        def s8_half(h, cc_sb):
            for c in range(4):
                ec = h * 4 + c
                ch = mv.tile([D, LH * 512], f8, tag="mv", name=f"hcm{ec}")
                nc.sync.dma_start(
                    out=ch[:],
                    in_=hcm_d.ap()[:, ec * LH * 512:(ec + 1) * LH * 512])
                po = ps_acc.tile([128, 512], fp32, tag="acc", name=f"p8_{ec}")
                for tau in range(LH):
                    nc.tensor.matmul(po[:], h1cv[:, tau * 128:(tau + 1) * 128],
                                     ch[:, tau * 512:(tau + 1) * 512],
                                     start=(tau == 0), stop=(tau == LH - 1))
                stg = ev.tile([128, 512], fp32, tag="ev", name=f"e8_{ec}")
                nc.vector.tensor_copy(stg[:], po[:])
                for q in range(4):
                    pt = ps_t.tile([128, 128], fp32, tag="pst",
                                   name=f"t8_{ec}_{q}")
                    nc.tensor.transpose(pt[:], stg[:, q * 128:(q + 1) * 128],
                                        ident[:])
                    col = (ec * 4 + q) * 128
                    nc.vector.tensor_copy(cc_sb[:, col:col + 128], pt[:])

        cc2_sb = one.tile([D, N], bf16, tag="cc2a")
        s8_half(0, cc2_sb)
        s8_half(1, cc2_sb)
        cc2_in = dram.tile([D, N], bf16, tag="cc2ai")
        cc2_out = dram.tile([D, N], bf16, tag="cc2ao")
        nc.sync.dma_start(out=cc2_in[:], in_=cc2_sb[:])
        nc.gpsimd.collective_compute(
            "AllReduce", Alu.add, replica_groups=GROUPS,
            ins=[cc2_in.opt()], outs=[cc2_out.opt()])

        # ---- S9: h1eT [128d, 2048s]; consume AR2 halves; + eps*hxT ---
        he_ps = [ps_acc.tile([128, 512], fp32, tag="acc", name=f"he{sc}")
                 for sc in range(4)]
        h1d_sb = one.tile([D, N], bf16, tag="h1bv", name="h1d_sb")
        nc.sync.dma_start(out=h1d_sb[:], in_=cc2_out[:])
        for eh in range(2):
            for sc in range(4):
                ch = mv.tile([D, LH * 512], bf16, tag="mv",
                             name=f"de{eh}_{sc}")
                off = (eh * 4 + sc) * LH * 512
                nc.sync.dma_start(out=ch[:],
                                  in_=det_d.ap()[:, off:off + LH * 512])
                for t in range(LH):
                    tg = eh * LH + t
                    nc.tensor.matmul(he_ps[sc][:],
                                     h1d_sb[:, tg * 128:(tg + 1) * 128],
                                     ch[:, t * 512:(t + 1) * 512],
                                     start=(tg == 0), stop=(tg == NT - 1))
        hT = one.tile([D, HS], fp32, tag="hT", name="hT")
        for sc in range(4):
            nc.vector.tensor_copy(hT[:, sc * 512:(sc + 1) * 512],
                                  he_ps[sc][:])
        hv = one.tile([128, HS], bf16, tag="hv")
        for t in range(LH):
            p = ps_t.tile([128, 128], fp32, tag="pst", name=f"t10{t}")
            nc.tensor.transpose(p[:], hT[:, t * 128:(t + 1) * 128], ident[:])
            nc.vector.tensor_copy(hv[:, t * 128:(t + 1) * 128], p[:])

        # ---- S11: out partial [n, d]; halves -> RSa/RSb --------------
        def s11_half(h, cc_in):
            for c in range(4):
                nc_i = h * 4 + c
                ch = mv.tile([D, LH * 512], f8, tag="mv", name=f"hbm2_{nc_i}")
                nc.sync.dma_start(
                    out=ch[:],
                    in_=hbm_d.ap()[:, nc_i * LH * 512:(nc_i + 1) * LH * 512])
                po = ps_acc.tile([128, 512], fp32, tag="acc",
                                 name=f"p11_{nc_i}")
                for j in range(LH):
                    nc.tensor.matmul(po[:], hv[:, j * 128:(j + 1) * 128],
                                     ch[:, j * 512:(j + 1) * 512],
                                     start=(j == 0), stop=(j == LH - 1))
                stg = ev.tile([128, 512], fp32, tag="ev", name=f"e11_{nc_i}")
                nc.vector.tensor_tensor(
                    stg[:], po[:], out2[:, nc_i * 512:(nc_i + 1) * 512],
                    op=Alu.add)
                for q in range(4):
                    pt = ps_t.tile([128, 128], fp32, tag="pst",
                                   name=f"t11_{nc_i}_{q}")
                    nc.tensor.transpose(pt[:], stg[:, q * 128:(q + 1) * 128],
                                        ident[:])
                    eo = ev.tile([128, 128], fp32, tag="ev2",
                                 name=f"eo11_{nc_i}_{q}")
                    nc.vector.tensor_copy(eo[:], pt[:])
                    row = (c * 4 + q) * 128
                    nc.sync.dma_start(out=cc_in.opt()[row:row + 128, :],
                                      in_=eo[:])

        cc4a_in = dram.tile([HS, D], fp32, tag="cc4ai")
        cc4a_out = dram.tile([HS // 2, D], fp32, tag="cc4ao")
        s11_half(0, cc4a_in)
        nc.gpsimd.collective_compute(
            "ReduceScatter", Alu.add, replica_groups=GROUPS,
            ins=[cc4a_in.opt()], outs=[cc4a_out.opt()])
        cc4b_in = dram.tile([HS, D], fp32, tag="cc4bi")
        cc4b_out = dram.tile([HS // 2, D], fp32, tag="cc4bo")
        s11_half(1, cc4b_in)
        nc.gpsimd.collective_compute(
            "ReduceScatter", Alu.add, replica_groups=GROUPS,
            ins=[cc4b_in.opt()], outs=[cc4b_out.opt()])

        # ---- S12: epilogue bn(lrelu(out)) per RS half ----------------
        y_sb = one.tile([D, HS], fp32, tag="y_sb")
        for h, cco in ((0, cc4a_out), (1, cc4b_out)):
            rs_sb = one.tile([128, HS // 2], fp32, tag="rs_sb",
                             name=f"rs{h}")
            nc.sync.dma_start(
                out=rs_sb[:].rearrange("p (t d) -> p t d", t=8),
                in_=cco.opt().rearrange("(t p) d -> p t d", p=128))
            for t in range(8):
                p = ps_t.tile([128, 128], fp32, tag="pst", name=f"t12{h}_{t}")
                nc.tensor.transpose(p[:], rs_sb[:, t * 128:(t + 1) * 128],
                                    ident[:])
                col = h * (HS // 2) + t * 128
                nc.scalar.activation(y_sb[:, col:col + 128], p[:],
                                     Act.Lrelu, alpha=0.01)
        nc.vector.tensor_scalar(y_sb[:], y_sb[:], sbn_t[:], tbn_t[:],
                                op0=Alu.mult, op1=Alu.add)
        nc.sync.dma_start(out=y_d.ap(), in_=y_sb[:])

    nc.finalize()
    return nc


def _get_nc():
    if "nc" not in _CACHE:
        _CACHE["nc"] = _build()
    return _CACHE["nc"]


def _rows(r):
    """Node rows owned by pair-role r: two interleaved 1024-blocks."""
    return np.r_[r * 1024:(r + 1) * 1024, 2048 + r * 1024:2048 + (r + 1) * 1024]


def _shard(inputs):
    f8 = ml_dtypes.float8_e4m3
    bf16 = ml_dtypes.bfloat16
    H = np.asarray(inputs["incident_mat"], dtype=np.float32)
    Dvm = np.asarray(inputs["degree_v"], dtype=np.float32)
    Dem = np.asarray(inputs["degree_e"], dtype=np.float32)
    x = np.asarray(inputs["x"], dtype=np.float32)
    em = np.asarray(inputs["e_masks"])
    W = np.asarray(inputs["mlp_W"], dtype=np.float32)
    bv = np.asarray(inputs["mlp_b"], dtype=np.float32)
    th = np.asarray(inputs["theta_att"], dtype=np.float32)
    eps = float(np.asarray(inputs["eps"]).reshape(-1)[0])
    bng = np.asarray(inputs["bn_gamma"], dtype=np.float32)
    bnb = np.asarray(inputs["bn_beta"], dtype=np.float32)
    bnm = np.asarray(inputs["bn_mean"], dtype=np.float32)
    bnv = np.asarray(inputs["bn_var"], dtype=np.float32)

    s_bn = (bng / np.sqrt(bnv + 1e-5)).reshape(D, 1).astype(np.float32)
    t_bn = (bnb - bnm * s_bn[:, 0]).reshape(D, 1).astype(np.float32)

    com = {
        "eps": np.full((D, 1), eps, dtype=np.float32),
        "sbn": np.ascontiguousarray(s_bn),
        "tbn": np.ascontiguousarray(t_bn),
    }
    xwv_g, attn_g = [], []
    for g in range(B):
        xw = (x[g] @ W + bv).astype(np.float32)
        xwv_g.append(np.ascontiguousarray(
            xw.reshape(NT, 128, D).transpose(1, 0, 2)
            .reshape(128, N).astype(bf16)))
        s = H[g].T @ (x[g] @ th[:, 0])
        s = np.where(em[g] == 0, -np.inf, s)
        a = np.exp(s - s.max())
        a = (a / a.sum()).astype(np.float32)
        attn_g.append(a)

    in_maps = []
    for c in range(NCORES):
        g, r = c // 2, c % 2
        e0 = r * HS
        rows = _rows(r)
        Hg = H[g].astype(f8)
        m = dict(com)
        m["xwv"] = xwv_g[g]
        m["attn"] = np.ascontiguousarray(attn_g[g][e0:e0 + HS].reshape(1, HS))
        # ha: [p, ec*NT*512 + k*512 + e_l] = H[k*128+p, e0 + ec*512 + e_l]
        A = Hg[:, e0:e0 + HS]
        m["ha"] = np.ascontiguousarray(
            A.reshape(NT, 128, 4, 512).transpose(1, 2, 0, 3)
            .reshape(128, 4 * NT * 512))
        # hbm: [p, nc*LH*512 + j*512 + n_l] = H[nc*512+n_l, e0 + j*128 + p]
        Bm = np.ascontiguousarray(A.T)       # [2048 e, 4096 n]
        m["hbm"] = np.ascontiguousarray(
            Bm.reshape(LH, 128, 8, 512).transpose(1, 2, 0, 3)
            .reshape(128, 8 * LH * 512))
        # hcm: [p, ec*LH*512 + tau*512 + e_l] = H[rows[tau*128+p], ec*512+e_l]
        Cm = Hg[rows, :]                     # [2048 n_l, 4096 e]
        m["hcm"] = np.ascontiguousarray(
            Cm.reshape(LH, 128, 8, 512).transpose(1, 2, 0, 3)
            .reshape(128, 8 * LH * 512))
        # dvt: [p, (kh*4+rc)*LH*512 + k*512 + r_l]
        #    = Dv[rows[rc*512+r_l], (kh*LH+k)*128 + p]
        Dm = np.ascontiguousarray(Dvm[g][rows, :].T.astype(bf16))  # [4096, 2048]
        m["dvt"] = np.ascontiguousarray(
            Dm.reshape(2, LH, 128, 4, 512).transpose(2, 0, 3, 1, 4)
            .reshape(128, NT * HS))
        # det: [p, (eh*4+sc)*LH*512 + t*512 + s_l]
        #    = De[e0 + sc*512 + s_l, (eh*LH+t)*128 + p]
        Em = np.ascontiguousarray(Dem[g][e0:e0 + HS, :].T.astype(bf16))
        m["det"] = np.ascontiguousarray(
            Em.reshape(2, LH, 128, 4, 512).transpose(2, 0, 3, 1, 4)
            .reshape(128, NT * HS))
        in_maps.append(m)
    return in_maps


def kernel(**inputs):
    from concourse.bass_utils import run_bass_kernel_spmd

    nc = _get_nc()
    in_maps = _shard(inputs)
    res = run_bass_kernel_spmd(nc, in_maps, list(range(NCORES)))
    out = np.empty((B, N, D), dtype=np.float32)
    for c in range(NCORES):
        g, r = c // 2, c % 2
        yc = res.results[c]["y"]             # [128 d, 2048] image
        blk = yc.reshape(D, LH, 128).transpose(1, 2, 0).reshape(HS, D)
        out[g, _rows(r), :] = blk
    return out
